# revision 13
# baseline (speedup 1.0000x reference)
"""Trainium2 Bass kernel for nn_MultiHeadDotProductAttention_14980845928960.

Block-local multi-head attention with partial RoPE:
  q/k/v projections -> RoPE on first 32 of 64 head dims -> softmax(QK^T/8)V
  -> output projection.  Shapes: inputs [4,16,256,1024], 16 heads x 64 dim,
  blocks of 256 tokens attend locally.

Strategy: data-parallel over the 64 (batch, block) pairs -> 8 blocks/core.
Per block, everything is computed with the contraction dim on SBUF
partitions:
  - x^T via PE transposes; projections as lhsT=W chunk, rhs=x^T chunk.
  - Q/K channel-PERMUTED (host side) so rope dims occupy out-chunks 0-3
    and pass dims chunks 4-7; RoPE = R-matmul (pair swap w/ signs) + two
    elementwise multiplies with cos/sin tables (position tables are
    host-precomputed inputs).
  - scores computed TRANSPOSED (k on partitions) so no P transpose is
    needed; softmax has no max-subtraction (scores ~N(0,1), exp is safe);
    row sums come free as a 65th output row of the PV matmul against
    v_aug = [v | 1] (interleaved 65-col layout); normalization folds into
    the attn PSUM->SBUF evacuation.
  - fp32r (reduced fp32, ~1e-4 rel) matmuls at bf16 rate for N>=256.
All matmul scaling (1/sqrt(D)) and biases fold into host-prepped weights
(bq,bk folded on evac; bv,bo folded as bo_eff = bo + bv @ Wo since
softmax rows sum to one).
"""

import numpy as np

import concourse.bass as bass
import concourse.tile as tile
from concourse import mybir
from concourse.bass_utils import run_bass_kernel_spmd
from concourse.vector_clock import ScopedClock

# ---------------------------------------------------------------- constants
B, NB, BS, F = 4, 16, 256, 1024
H, D, ROPE = 16, 64, 32
NCORES = 8
BLKS = B * NB                 # 64 blocks total
BPC = BLKS // NCORES          # 8 blocks per core
F32 = mybir.dt.float32
F32R = mybir.dt.float32r
MULT = mybir.AluOpType.mult
ADD = mybir.AluOpType.add
EXP = mybir.ActivationFunctionType.Exp

# ------------------------------------------------- walrus multi-wait splitter
# This walrus build rejects >1 sync-wait per instruction on several
# instruction structs. Tile attaches several waits to one instruction;
# hoist extras onto NOPs inserted just before it on the same engine.
_split_ctr = [0]


def _split_multi_waits(nc, maxw=1):
    for f in nc.m.functions:
        for bb in f.blocks:
            insts = list(bb.instructions)
            out = []
            changed = False
            for inst in insts:
                si = inst.sync_info
                waits = list(si.on_wait) if si and si.on_wait else []
                if len(waits) > maxw:
                    changed = True
                    for w in waits[:-maxw]:
                        _split_ctr[0] += 1
                        nop = mybir.InstNoOp(
                            name=f"wsplit-{_split_ctr[0]}",
                            ins=[],
                            outs=[],
                            engine=inst.engine,
                        )
                        nop.sync_info = mybir.SyncInfo(on_wait=[w], on_update=[])
                        nc.register_instruction(nop)
                        out.append(nop)
                    si.on_wait = waits[-maxw:]
                out.append(inst)
            if changed:
                bb.instructions = out


# ---------------------------------------------------------------- bass build
def _build_block(nc, pools, consts, blk, dram):
    """Emit one (batch, block) tile of work."""
    psum, xin, xt, qk, vpool, ptp, attnp, outp, tabp = pools
    wq_sb, wk_sb, wv_sb, wo_sb, rt_sb, ident, bq_sb, bk_sb, bo_sb, vaug = consts
    xq_d, xkv_d, cos_d, sin_d, out_d = dram

    cos_sb = tabp.tile([128, 256], F32, tag="cos")
    nc.sync.dma_start(out=cos_sb, in_=cos_d[blk])
    sin_sb = tabp.tile([128, 256], F32, tag="sin")
    nc.sync.dma_start(out=sin_sb, in_=sin_d[blk])

    # ---- x^T for both inputs: 8 f-chunks of [128f, 256tok] each
    def transpose_input(x_d, tagpfx):
        tiles = []
        for c in range(8):
            ps = psum.tile([128, 256], F32R, tag="ps")
            for t in range(2):
                xtile = xin.tile([128, 128], F32R, tag="xin")
                nc.gpsimd.dma_start(
                    out=xtile,
                    in_=x_d[blk, t * 128 : (t + 1) * 128, c * 128 : (c + 1) * 128],
                )
                nc.tensor.transpose(
                    out=ps[:, t * 128 : (t + 1) * 128], in_=xtile, identity=ident
                )
            tt = xt.tile([128, 256], F32R, tag=f"{tagpfx}{c}")
            nc.any.tensor_copy(out=tt, in_=ps)
            tiles.append(tt)
        return tiles


    # ---- Q / K projections (channel-permuted; chunks 0-3 rope, 4-7 pass)
    def qk_proj(w_sb, b_sb, x_tiles, tagpfx):
        outs = []
        for oc in range(8):
            ps = psum.tile([128, 256], F32, tag="ps")
            for c in range(8):
                nc.tensor.matmul(
                    ps,
                    lhsT=w_sb[c][:, oc * 128 : (oc + 1) * 128],
                    rhs=x_tiles[c],
                    start=(c == 0),
                    stop=(c == 7),
                )
            qf = qk.tile([128, 256], F32R, tag=f"{tagpfx}{oc}")
            if oc < 4:
                raw = qk.tile([128, 256], F32R, tag="raw", bufs=2)
                nc.vector.tensor_scalar_add(raw, ps, b_sb[:, oc : oc + 1])
                ps2 = psum.tile([128, 256], F32, tag="ps")
                nc.tensor.matmul(ps2, lhsT=rt_sb, rhs=raw, start=True, stop=True)
                qs2 = qk.tile([128, 256], F32, tag="qs2", bufs=2)
                nc.vector.tensor_tensor(out=qs2, in0=ps2, in1=sin_sb, op=MULT)
                nc.any.tensor_tensor(out=qf, in0=raw, in1=cos_sb, op=MULT)
                nc.any.tensor_tensor(out=qf, in0=qf, in1=qs2, op=ADD)
            else:
                nc.vector.tensor_scalar_add(qf, ps, b_sb[:, oc : oc + 1])
            outs.append(qf)
        return outs

    xqT = transpose_input(xq_d, "xt")
    qT = qk_proj(wq_sb, bq_sb, xqT, "q")
    xkT = transpose_input(xkv_d, "xt")
    kT = qk_proj(wk_sb, bk_sb, xkT, "k")

    # ---- V projection into interleaved v_aug = [v_h | 1]*64 (128 cols/head)
    # The 64 ones-columns replicate the softmax row-sum onto PV output
    # partitions 64..127, already partition-broadcast for normalization.
    for kc in range(2):
        va = vaug[kc]
        va3 = va.rearrange("p (h c) -> p h c", c=128)
        for b2 in range(2):
            ps = psum.tile([128, 512], F32, tag="ps")
            for c in range(8):
                nc.tensor.matmul(
                    ps,
                    lhsT=xkT[c][:, kc * 128 : (kc + 1) * 128],
                    rhs=wv_sb[c][:, b2 * 512 : (b2 + 1) * 512],
                    start=(c == 0),
                    stop=(c == 7),
                )
            nc.any.tensor_copy(
                out=va3[:, b2 * 8 : (b2 + 1) * 8, 0:64],
                in_=ps.rearrange("p (h c) -> p h c", c=64),
            )

    # ---- attention (scores transposed: [k, q]; 4 heads per chunk-group)
    attnT = [
        attnp.tile([128, 256], F32R, tag=f"attnT{cc}", name=f"attnT{cc}")
        for cc in range(8)
    ]
    for hg in range(4):
        rc, pc = hg, 4 + hg
        pts = {}
        for kc in range(2):
            sps = []
            for g in range(4):
                ps = psum.tile([128, 256], F32, tag="ps")
                r0 = 32 * g
                nc.tensor.matmul(
                    ps,
                    lhsT=kT[rc][r0 : r0 + 32, kc * 128 : (kc + 1) * 128],
                    rhs=qT[rc][r0 : r0 + 32, :],
                    start=True,
                    stop=False,
                    tile_position=(r0, 0),
                )
                nc.tensor.matmul(
                    ps,
                    lhsT=kT[pc][r0 : r0 + 32, kc * 128 : (kc + 1) * 128],
                    rhs=qT[pc][r0 : r0 + 32, :],
                    start=False,
                    stop=True,
                    tile_position=(r0, 0),
                )
                sps.append(ps)
            for g in range(4):
                pt = ptp.tile([128, 256], F32R, tag="pt")
                nc.scalar.activation(out=pt, in_=sps[g], func=EXP)
                pts[(g, kc)] = pt
        for g in range(4):
            h = 4 * hg + g
            aps = psum.tile([128, 256], F32, tag="ps")
            for kc in range(2):
                nc.tensor.matmul(
                    aps,
                    lhsT=vaug[kc][:, h * 128 : (h + 1) * 128],
                    rhs=pts[(g, kc)],
                    start=(kc == 0),
                    stop=(kc == 1),
                )
            rec_b = attnp.tile([64, 256], F32, tag="recip", bufs=2)
            nc.vector.reciprocal(out=rec_b, in_=aps[64:128, :])
            cc, r0 = h // 2, (h % 2) * 64
            nc.vector.tensor_tensor(
                out=attnT[cc][r0 : r0 + 64, :],
                in0=aps[0:64, :],
                in1=rec_b,
                op=MULT,
            )

    # ---- output projection + bias
    for t2 in range(2):
        for n2 in range(2):
            ps = psum.tile([128, 512], F32, tag="ps")
            for cc in range(8):
                nc.tensor.matmul(
                    ps,
                    lhsT=attnT[cc][:, t2 * 128 : (t2 + 1) * 128],
                    rhs=wo_sb[cc][:, n2 * 512 : (n2 + 1) * 512],
                    start=(cc == 0),
                    stop=(cc == 7),
                )
            ob = outp.tile([128, 512], F32, tag="outsb")
            nc.vector.tensor_tensor(
                out=ob,
                in0=ps,
                in1=bo_sb[:, n2 * 512 : (n2 + 1) * 512],
                op=ADD,
            )
            nc.sync.dma_start(
                out=out_d[blk, t2 * 128 : (t2 + 1) * 128, n2 * 512 : (n2 + 1) * 512],
                in_=ob,
            )


def build_program():
    nc = bass.Bass("TRN2")
    xq_d = nc.dram_tensor("xq", [BPC, BS, F], F32R, kind="ExternalInput")
    xkv_d = nc.dram_tensor("xkv", [BPC, BS, F], F32R, kind="ExternalInput")
    wq_d = nc.dram_tensor("wq", [8, 128, F], F32R, kind="ExternalInput")
    wk_d = nc.dram_tensor("wk", [8, 128, F], F32R, kind="ExternalInput")
    wv_d = nc.dram_tensor("wv", [8, 128, F], F32R, kind="ExternalInput")
    wo_d = nc.dram_tensor("wo", [8, 128, F], F32R, kind="ExternalInput")
    rt_d = nc.dram_tensor("rt", [128, 128], F32R, kind="ExternalInput")
    ident_d = nc.dram_tensor("ident", [128, 128], F32R, kind="ExternalInput")
    ones_d = nc.dram_tensor("ones", [1, 16, 64], F32R, kind="ExternalInput")
    bq_d = nc.dram_tensor("bq", [128, 8], F32, kind="ExternalInput")
    bk_d = nc.dram_tensor("bk", [128, 8], F32, kind="ExternalInput")
    bo_d = nc.dram_tensor("bo", [1, F], F32, kind="ExternalInput")
    cos_d = nc.dram_tensor("cos", [BPC, 128, 256], F32, kind="ExternalInput")
    sin_d = nc.dram_tensor("sin", [BPC, 128, 256], F32, kind="ExternalInput")
    out_d = nc.dram_tensor("out", [BPC, BS, F], F32, kind="ExternalOutput")

    with tile.TileContext(nc) as tc:
        with (
            tc.tile_pool(name="wpool", bufs=1) as wpool,
            tc.tile_pool(name="psum", bufs=8, space="PSUM") as psum,
            tc.tile_pool(name="xin", bufs=4) as xin,
            tc.tile_pool(name="xt", bufs=1) as xt,
            tc.tile_pool(name="qk", bufs=1) as qk,
            tc.tile_pool(name="vpool", bufs=1) as vpool,
            tc.tile_pool(name="ptp", bufs=8) as ptp,
            tc.tile_pool(name="attnp", bufs=1) as attnp,
            tc.tile_pool(name="outp", bufs=2) as outp,
            tc.tile_pool(name="tabp", bufs=2) as tabp,
        ):
            def wtiles(src, tagpfx):
                ts = []
                for c in range(8):
                    t = wpool.tile([128, F], F32R, tag=f"{tagpfx}{c}")
                    nc.sync.dma_start(out=t, in_=src[c])
                    ts.append(t)
                return ts

            wq_sb = wtiles(wq_d, "wq")
            wk_sb = wtiles(wk_d, "wk")
            wv_sb = wtiles(wv_d, "wv")
            wo_sb = wtiles(wo_d, "wo")
            rt_sb = wpool.tile([128, 128], F32R, tag="rt")
            nc.sync.dma_start(out=rt_sb, in_=rt_d[:])
            ident = wpool.tile([128, 128], F32R, tag="ident")
            nc.sync.dma_start(out=ident, in_=ident_d[:])
            bq_sb = wpool.tile([128, 8], F32, tag="bq")
            nc.sync.dma_start(out=bq_sb, in_=bq_d[:])
            bk_sb = wpool.tile([128, 8], F32, tag="bk")
            nc.sync.dma_start(out=bk_sb, in_=bk_d[:])
            bo_sb = wpool.tile([128, F], F32, tag="bo")
            nc.sync.dma_start(out=bo_sb, in_=bo_d[0:1, :].to_broadcast([128, F]))

            vaug = []
            for kc in range(2):
                va = vpool.tile([128, 2048], F32R, tag=f"vaug{kc}", name=f"vaug{kc}")
                nc.sync.dma_start(
                    out=va.rearrange("p (h c) -> p h c", c=128)[:, :, 64:128],
                    in_=ones_d[:].to_broadcast([128, 16, 64]),
                )
                vaug.append(va)

            pools = (psum, xin, xt, qk, vpool, ptp, attnp, outp, tabp)
            consts = (wq_sb, wk_sb, wv_sb, wo_sb, rt_sb, ident, bq_sb, bk_sb, bo_sb, vaug)
            dram = (xq_d, xkv_d, cos_d, sin_d, out_d)
            for blk in range(BPC):
                _build_block(nc, pools, consts, blk, dram)

    _split_multi_waits(nc)
    return nc


# ---------------------------------------------------------------- host side
def _host_prep(Wq, bq, Wk, bk, Wv, bv, Wo, bo):
    """Permute/scale weights; fold biases."""
    old_of_new = np.empty(F, np.int64)
    for h in range(H):
        old_of_new[h * ROPE : (h + 1) * ROPE] = h * D + np.arange(ROPE)
        old_of_new[512 + h * ROPE : 512 + (h + 1) * ROPE] = (
            h * D + ROPE + np.arange(ROPE)
        )
    wq_flat = (Wq.reshape(F, F) / np.sqrt(D)).astype(np.float32)
    wq_p = np.ascontiguousarray(wq_flat[:, old_of_new]).reshape(8, 128, F)
    wk_flat = Wk.reshape(F, F).astype(np.float32)
    wk_p = np.ascontiguousarray(wk_flat[:, old_of_new]).reshape(8, 128, F)
    wv_c = np.ascontiguousarray(Wv.reshape(F, F)).reshape(8, 128, F)
    wo_c = np.ascontiguousarray(Wo.reshape(F, F)).reshape(8, 128, F)
    bq_p = np.ascontiguousarray(
        (bq.reshape(F) / np.sqrt(D))[old_of_new].reshape(8, 128).T
    ).astype(np.float32)
    bk_p = np.ascontiguousarray(bk.reshape(F)[old_of_new].reshape(8, 128).T).astype(
        np.float32
    )
    bo_eff = (bo + bv.reshape(F) @ Wo.reshape(F, F)).reshape(1, F).astype(np.float32)

    # R^T for rotate_every_two with signs: (R@q)[2i] = -q[2i+1]; [2i+1] = q[2i]
    R = np.zeros((128, 128), np.float32)
    for g in range(4):          # 4 heads per rope chunk, 32 rows each
        for i in range(ROPE // 2):
            R[g * 32 + 2 * i, g * 32 + 2 * i + 1] = -1.0
            R[g * 32 + 2 * i + 1, g * 32 + 2 * i] = 1.0
    rt = np.ascontiguousarray(R.T)
    return wq_p, wk_p, wv_c, wo_c, bq_p, bk_p, bo_eff, rt


def _tables_for_core(core):
    """cos/sin tables [BPC, 128, 256] for this core's blocks."""
    inv_freq = (1.0 / 10000.0 ** (np.arange(0, ROPE, 2) / ROPE)).astype(np.float64)
    cos_t = np.empty((BPC, 128, 256), np.float32)
    sin_t = np.empty((BPC, 128, 256), np.float32)
    for i in range(BPC):
        nb = (core * BPC + i) % NB
        pos = nb * BS + np.arange(BS, dtype=np.float64)
        ang = pos[None, :] * inv_freq[:, None]          # [16, 256]
        cpat = np.repeat(np.cos(ang), 2, axis=0)        # [32, 256]
        spat = np.repeat(np.sin(ang), 2, axis=0)
        cos_t[i] = np.tile(cpat, (4, 1)).astype(np.float32)
        sin_t[i] = np.tile(spat, (4, 1)).astype(np.float32)
    return cos_t, sin_t


_nc_cache = []


def kernel(inputs_q, inputs_kv, Wq, bq, Wk, bk, Wv, bv, Wo, bo):
    inputs_q = np.asarray(inputs_q, np.float32)
    inputs_kv = np.asarray(inputs_kv, np.float32)
    wq_p, wk_p, wv_c, wo_c, bq_p, bk_p, bo_eff, rt = _host_prep(
        np.asarray(Wq), np.asarray(bq), np.asarray(Wk), np.asarray(bk),
        np.asarray(Wv), np.asarray(bv), np.asarray(Wo), np.asarray(bo),
    )
    xq_all = inputs_q.reshape(BLKS, BS, F)
    xkv_all = inputs_kv.reshape(BLKS, BS, F)

    if not _nc_cache:
        _nc_cache.append(build_program())
    nc = _nc_cache[0]

    in_maps = []
    for core in range(NCORES):
        cos_t, sin_t = _tables_for_core(core)
        in_maps.append(
            {
                "xq": np.ascontiguousarray(xq_all[core * BPC : (core + 1) * BPC]),
                "xkv": np.ascontiguousarray(xkv_all[core * BPC : (core + 1) * BPC]),
                "wq": wq_p, "wk": wk_p, "wv": wv_c, "wo": wo_c,
                "rt": rt, "bq": bq_p, "bk": bk_p, "bo": bo_eff,
                "ident": np.eye(128, dtype=np.float32),
                "ones": np.ones((1, 16, 64), np.float32),
                "cos": cos_t, "sin": sin_t,
            }
        )
    res = run_bass_kernel_spmd(nc, in_maps, list(range(NCORES)))
    out = np.concatenate([res.results[i]["out"] for i in range(NCORES)], axis=0)
    return out.reshape(B, NB, BS, F)


# revision 14
# speedup vs baseline: 1.3690x; 1.3690x over previous
"""Trainium2 Bass kernel for nn_MultiHeadDotProductAttention_14980845928960.

Block-local multi-head attention with partial RoPE:
  q/k/v projections -> RoPE on first 32 of 64 head dims -> softmax(QK^T/8)V
  -> output projection.  Shapes: inputs [4,16,256,1024], 16 heads x 64 dim,
  blocks of 256 tokens attend locally.

Strategy: data-parallel over the 64 (batch, block) pairs -> 8 blocks/core.
Per block, everything is computed with the contraction dim on SBUF
partitions:
  - x^T via PE transposes; projections as lhsT=W chunk, rhs=x^T chunk.
  - Q/K channel-PERMUTED (host side) so rope dims occupy out-chunks 0-3
    and pass dims chunks 4-7; RoPE = R-matmul (pair swap w/ signs) + two
    elementwise multiplies with cos/sin tables (position tables are
    host-precomputed inputs).
  - scores computed TRANSPOSED (k on partitions) so no P transpose is
    needed; softmax has no max-subtraction (scores ~N(0,1), exp is safe);
    row sums come free as a 65th output row of the PV matmul against
    v_aug = [v | 1] (interleaved 65-col layout); normalization folds into
    the attn PSUM->SBUF evacuation.
  - fp32r (reduced fp32, ~1e-4 rel) matmuls at bf16 rate for N>=256.
All matmul scaling (1/sqrt(D)) and biases fold into host-prepped weights
(bq,bk folded on evac; bv,bo folded as bo_eff = bo + bv @ Wo since
softmax rows sum to one).
"""

import ml_dtypes
import numpy as np

import concourse.bass as bass
import concourse.tile as tile
from concourse import mybir
from concourse.bass_utils import run_bass_kernel_spmd
from concourse.vector_clock import ScopedClock

# ---------------------------------------------------------------- constants
B, NB, BS, F = 4, 16, 256, 1024
H, D, ROPE = 16, 64, 32
NCORES = 8
BLKS = B * NB                 # 64 blocks total
BPC = BLKS // NCORES          # 8 blocks per core
F32 = mybir.dt.float32
F32R = mybir.dt.float32r
BF16 = mybir.dt.bfloat16
WDT = BF16                    # projection-weight / xT / attnT dtype
WNP = ml_dtypes.bfloat16
MULT = mybir.AluOpType.mult
ADD = mybir.AluOpType.add
EXP = mybir.ActivationFunctionType.Exp

# ------------------------------------------------- walrus multi-wait splitter
# This walrus build rejects >1 sync-wait per instruction on several
# instruction structs. Tile attaches several waits to one instruction;
# hoist extras onto NOPs inserted just before it on the same engine.
_split_ctr = [0]


def _split_multi_waits(nc, maxw=1):
    for f in nc.m.functions:
        for bb in f.blocks:
            insts = list(bb.instructions)
            out = []
            changed = False
            for inst in insts:
                si = inst.sync_info
                waits = list(si.on_wait) if si and si.on_wait else []
                if len(waits) > maxw:
                    changed = True
                    for w in waits[:-maxw]:
                        _split_ctr[0] += 1
                        nop = mybir.InstNoOp(
                            name=f"wsplit-{_split_ctr[0]}",
                            ins=[],
                            outs=[],
                            engine=inst.engine,
                        )
                        nop.sync_info = mybir.SyncInfo(on_wait=[w], on_update=[])
                        nc.register_instruction(nop)
                        out.append(nop)
                    si.on_wait = waits[-maxw:]
                out.append(inst)
            if changed:
                bb.instructions = out


def _act_reciprocal(nc, out, in_):
    # ScalarE LUT reciprocal (~1.2e-5 rel, 507ns/[64,256]) -- bass's guard
    # prefers DVE reciprocal, which is 3.3x slower; emit directly.
    eng = nc.scalar
    return eng.add_instruction(
        mybir.InstActivation(
            name=nc.get_next_instruction_name(),
            func=mybir.ActivationFunctionType.Reciprocal,
            ins=[
                eng.lower_ap(in_),
                mybir.ImmediateValue(dtype=F32, value=0.0),
                mybir.ImmediateValue(dtype=F32, value=1.0),
                mybir.ImmediateValue(dtype=F32, value=0.0),
            ],
            outs=[eng.lower_ap(out)],
        )
    )


# ---------------------------------------------------------------- bass build
def _build_block(nc, pools, consts, blk, dram):
    """Emit one (batch, block) tile of work."""
    psum, xin, xt, qk, vpool, ptp, attnp, outp, tabp = pools
    wq_sb, wk_sb, wv_sb, wo_sb, rt_sb, ident, bq_sb, bk_sb, bo_sb, vaug = consts
    xq_d, xkv_d, cos_d, sin_d, out_d = dram

    cos_sb = tabp.tile([128, 256], F32, tag="cos")
    nc.sync.dma_start(out=cos_sb, in_=cos_d[blk])
    sin_sb = tabp.tile([128, 256], F32, tag="sin")
    nc.sync.dma_start(out=sin_sb, in_=sin_d[blk])

    # ---- x^T for both inputs: 8 f-chunks of [128f, 256tok] each
    def transpose_input(x_d, dma_eng):
        # load as 4 [128, 512] tiles (2KB contiguous lines), transpose
        # [128,128] slices out of them
        xt_in = {}
        for t in range(2):
            for fh in range(2):
                xtile = xin.tile([128, 512], F32R, tag="xin", name=f"xin{t}{fh}")
                dma_eng.dma_start(
                    out=xtile,
                    in_=x_d[blk, t * 128 : (t + 1) * 128, fh * 512 : (fh + 1) * 512],
                )
                xt_in[(t, fh)] = xtile
        tiles = []
        for c in range(8):
            ps = psum.tile([128, 256], F32R, tag="ps")
            for t in range(2):
                src = xt_in[(t, c // 4)][:, (c % 4) * 128 : (c % 4 + 1) * 128]
                nc.tensor.transpose(
                    out=ps[:, t * 128 : (t + 1) * 128], in_=src, identity=ident
                )
            tt = xt.tile([128, 256], WDT, tag=f"xt{c}")
            nc.any.tensor_copy(out=tt, in_=ps)
            tiles.append(tt)
        return tiles


    # ---- Q / K projections (channel-permuted; chunks 0-3 rope, 4-7 pass)
    def qk_proj(w_sb, b_sb, x_tiles, tagpfx):
        outs = []
        for oc in range(8):
            ps = psum.tile([128, 256], F32, tag="ps")
            for c in range(8):
                nc.tensor.matmul(
                    ps,
                    lhsT=w_sb[c][:, oc * 128 : (oc + 1) * 128],
                    rhs=x_tiles[c],
                    start=(c == 0),
                    stop=(c == 7),
                )
            qf = qk.tile([128, 256], F32R, tag=f"{tagpfx}{oc}")
            if oc < 4:
                raw = qk.tile([128, 256], F32R, tag="raw", bufs=2)
                nc.vector.tensor_scalar_add(raw, ps, b_sb[:, oc : oc + 1])
                ps2 = psum.tile([128, 256], F32, tag="ps")
                nc.tensor.matmul(ps2, lhsT=rt_sb, rhs=raw, start=True, stop=True)
                qs2 = qk.tile([128, 256], F32, tag="qs2", bufs=2)
                nc.vector.tensor_tensor(out=qs2, in0=ps2, in1=sin_sb, op=MULT)
                nc.gpsimd.tensor_tensor(out=qf, in0=raw, in1=cos_sb, op=MULT)
                nc.gpsimd.tensor_tensor(out=qf, in0=qf, in1=qs2, op=ADD)
            else:
                nc.vector.tensor_scalar_add(qf, ps, b_sb[:, oc : oc + 1])
            outs.append(qf)
        return outs

    xqT = transpose_input(xq_d, nc.gpsimd)
    qT = qk_proj(wq_sb, bq_sb, xqT, "q")
    xkT = transpose_input(xkv_d, nc.scalar)
    kT = qk_proj(wk_sb, bk_sb, xkT, "k")

    # ---- V projection into interleaved v_aug = [v_h | 1]*64 (128 cols/head)
    # The 64 ones-columns replicate the softmax row-sum onto PV output
    # partitions 64..127, already partition-broadcast for normalization.
    for kc in range(2):
        va = vaug[kc]
        va3 = va.rearrange("p (h c) -> p h c", c=128)
        for b2 in range(2):
            ps = psum.tile([128, 512], F32, tag="ps")
            for c in range(8):
                nc.tensor.matmul(
                    ps,
                    lhsT=xkT[c][:, kc * 128 : (kc + 1) * 128],
                    rhs=wv_sb[c][:, b2 * 512 : (b2 + 1) * 512],
                    start=(c == 0),
                    stop=(c == 7),
                )
            nc.any.tensor_copy(
                out=va3[:, b2 * 8 : (b2 + 1) * 8, 0:64],
                in_=ps.rearrange("p (h c) -> p h c", c=64),
            )

    # ---- attention (scores transposed: [k, q]; 4 heads per chunk-group)
    attnT = [
        attnp.tile([128, 256], WDT, tag=f"attnT{cc}", name=f"attnT{cc}", bufs=2)
        for cc in range(8)
    ]
    for hg in range(4):
        rc, pc = hg, 4 + hg
        pts = {}
        for kc in range(2):
            sps = []
            for g in range(4):
                ps = psum.tile([128, 256], F32, tag="ps")
                r0 = 32 * g
                nc.tensor.matmul(
                    ps,
                    lhsT=kT[rc][r0 : r0 + 32, kc * 128 : (kc + 1) * 128],
                    rhs=qT[rc][r0 : r0 + 32, :],
                    start=True,
                    stop=False,
                    tile_position=(r0, 0),
                )
                nc.tensor.matmul(
                    ps,
                    lhsT=kT[pc][r0 : r0 + 32, kc * 128 : (kc + 1) * 128],
                    rhs=qT[pc][r0 : r0 + 32, :],
                    start=False,
                    stop=True,
                    tile_position=(r0, 0),
                )
                sps.append(ps)
            for g in range(4):
                pt = ptp.tile([128, 256], F32R, tag="pt")
                nc.scalar.activation(out=pt, in_=sps[g], func=EXP)
                pts[(g, kc)] = pt
        for g in range(4):
            h = 4 * hg + g
            aps = psum.tile([128, 256], F32, tag="ps")
            for kc in range(2):
                nc.tensor.matmul(
                    aps,
                    lhsT=vaug[kc][:, h * 128 : (h + 1) * 128],
                    rhs=pts[(g, kc)],
                    start=(kc == 0),
                    stop=(kc == 1),
                )
            rec_b = attnp.tile([64, 256], F32, tag="recip", bufs=2)
            _act_reciprocal(nc, rec_b, aps[64:128, :])
            cc, r0 = h // 2, (h % 2) * 64
            nc.vector.tensor_tensor(
                out=attnT[cc][r0 : r0 + 64, :],
                in0=aps[0:64, :],
                in1=rec_b,
                op=MULT,
            )

    # ---- output projection + bias
    for t2 in range(2):
        for n2 in range(2):
            ps = psum.tile([128, 512], F32, tag="ps")
            for cc in range(8):
                nc.tensor.matmul(
                    ps,
                    lhsT=attnT[cc][:, t2 * 128 : (t2 + 1) * 128],
                    rhs=wo_sb[cc][:, n2 * 512 : (n2 + 1) * 512],
                    start=(cc == 0),
                    stop=(cc == 7),
                )
            ob = outp.tile([128, 512], F32, tag="outsb")
            nc.vector.tensor_tensor(
                out=ob,
                in0=ps,
                in1=bo_sb[:, n2 * 512 : (n2 + 1) * 512],
                op=ADD,
            )
            nc.sync.dma_start(
                out=out_d[blk, t2 * 128 : (t2 + 1) * 128, n2 * 512 : (n2 + 1) * 512],
                in_=ob,
            )


def build_program():
    nc = bass.Bass("TRN2")
    xq_d = nc.dram_tensor("xq", [BPC, BS, F], F32R, kind="ExternalInput")
    xkv_d = nc.dram_tensor("xkv", [BPC, BS, F], F32R, kind="ExternalInput")
    wq_d = nc.dram_tensor("wq", [8, 128, F], WDT, kind="ExternalInput")
    wk_d = nc.dram_tensor("wk", [8, 128, F], WDT, kind="ExternalInput")
    wv_d = nc.dram_tensor("wv", [8, 128, F], WDT, kind="ExternalInput")
    wo_d = nc.dram_tensor("wo", [8, 128, F], WDT, kind="ExternalInput")
    rt_d = nc.dram_tensor("rt", [128, 128], F32R, kind="ExternalInput")
    ident_d = nc.dram_tensor("ident", [128, 128], F32R, kind="ExternalInput")
    ones_d = nc.dram_tensor("ones", [1, 16, 64], F32R, kind="ExternalInput")
    bq_d = nc.dram_tensor("bq", [128, 8], F32, kind="ExternalInput")
    bk_d = nc.dram_tensor("bk", [128, 8], F32, kind="ExternalInput")
    bo_d = nc.dram_tensor("bo", [1, F], F32, kind="ExternalInput")
    cos_d = nc.dram_tensor("cos", [BPC, 128, 256], F32, kind="ExternalInput")
    sin_d = nc.dram_tensor("sin", [BPC, 128, 256], F32, kind="ExternalInput")
    out_d = nc.dram_tensor("out", [BPC, BS, F], F32, kind="ExternalOutput")

    with tile.TileContext(nc) as tc:
        with (
            tc.tile_pool(name="wpool", bufs=1) as wpool,
            tc.tile_pool(name="psum", bufs=8, space="PSUM") as psum,
            tc.tile_pool(name="xin", bufs=4) as xin,
            tc.tile_pool(name="xt", bufs=1) as xt,
            tc.tile_pool(name="qk", bufs=2) as qk,
            tc.tile_pool(name="vpool", bufs=1) as vpool,
            tc.tile_pool(name="ptp", bufs=8) as ptp,
            tc.tile_pool(name="attnp", bufs=1) as attnp,
            tc.tile_pool(name="outp", bufs=2) as outp,
            tc.tile_pool(name="tabp", bufs=2) as tabp,
        ):
            def wtiles(src, tagpfx):
                ts = []
                for c in range(8):
                    t = wpool.tile([128, F], WDT, tag=f"{tagpfx}{c}")
                    nc.sync.dma_start(out=t, in_=src[c])
                    ts.append(t)
                return ts

            wq_sb = wtiles(wq_d, "wq")
            wk_sb = wtiles(wk_d, "wk")
            wv_sb = wtiles(wv_d, "wv")
            wo_sb = wtiles(wo_d, "wo")
            rt_sb = wpool.tile([128, 128], F32R, tag="rt")
            nc.sync.dma_start(out=rt_sb, in_=rt_d[:])
            ident = wpool.tile([128, 128], F32R, tag="ident")
            nc.sync.dma_start(out=ident, in_=ident_d[:])
            bq_sb = wpool.tile([128, 8], F32, tag="bq")
            nc.sync.dma_start(out=bq_sb, in_=bq_d[:])
            bk_sb = wpool.tile([128, 8], F32, tag="bk")
            nc.sync.dma_start(out=bk_sb, in_=bk_d[:])
            bo_sb = wpool.tile([128, F], F32, tag="bo")
            nc.sync.dma_start(out=bo_sb, in_=bo_d[0:1, :].to_broadcast([128, F]))

            vaug = []
            for kc in range(2):
                va = vpool.tile([128, 2048], F32R, tag=f"vaug{kc}", name=f"vaug{kc}")
                nc.sync.dma_start(
                    out=va.rearrange("p (h c) -> p h c", c=128)[:, :, 64:128],
                    in_=ones_d[:].to_broadcast([128, 16, 64]),
                )
                vaug.append(va)

            pools = (psum, xin, xt, qk, vpool, ptp, attnp, outp, tabp)
            consts = (wq_sb, wk_sb, wv_sb, wo_sb, rt_sb, ident, bq_sb, bk_sb, bo_sb, vaug)
            dram = (xq_d, xkv_d, cos_d, sin_d, out_d)
            for blk in range(BPC):
                _build_block(nc, pools, consts, blk, dram)

    _split_multi_waits(nc)
    return nc


# ---------------------------------------------------------------- host side
def _host_prep(Wq, bq, Wk, bk, Wv, bv, Wo, bo):
    """Permute/scale weights; fold biases."""
    old_of_new = np.empty(F, np.int64)
    for h in range(H):
        old_of_new[h * ROPE : (h + 1) * ROPE] = h * D + np.arange(ROPE)
        old_of_new[512 + h * ROPE : 512 + (h + 1) * ROPE] = (
            h * D + ROPE + np.arange(ROPE)
        )
    wq_flat = (Wq.reshape(F, F) / np.sqrt(D)).astype(np.float32)
    wq_p = np.ascontiguousarray(wq_flat[:, old_of_new]).reshape(8, 128, F)
    wk_flat = Wk.reshape(F, F).astype(np.float32)
    wk_p = np.ascontiguousarray(wk_flat[:, old_of_new]).reshape(8, 128, F)
    wv_c = np.ascontiguousarray(Wv.reshape(F, F)).reshape(8, 128, F)
    wo_c = np.ascontiguousarray(Wo.reshape(F, F)).reshape(8, 128, F)
    bq_p = np.ascontiguousarray(
        (bq.reshape(F) / np.sqrt(D))[old_of_new].reshape(8, 128).T
    ).astype(np.float32)
    bk_p = np.ascontiguousarray(bk.reshape(F)[old_of_new].reshape(8, 128).T).astype(
        np.float32
    )
    bo_eff = (bo + bv.reshape(F) @ Wo.reshape(F, F)).reshape(1, F).astype(np.float32)

    # R^T for rotate_every_two with signs: (R@q)[2i] = -q[2i+1]; [2i+1] = q[2i]
    R = np.zeros((128, 128), np.float32)
    for g in range(4):          # 4 heads per rope chunk, 32 rows each
        for i in range(ROPE // 2):
            R[g * 32 + 2 * i, g * 32 + 2 * i + 1] = -1.0
            R[g * 32 + 2 * i + 1, g * 32 + 2 * i] = 1.0
    rt = np.ascontiguousarray(R.T)
    return wq_p, wk_p, wv_c, wo_c, bq_p, bk_p, bo_eff, rt


def _tables_for_core(core):
    """cos/sin tables [BPC, 128, 256] for this core's blocks."""
    inv_freq = (1.0 / 10000.0 ** (np.arange(0, ROPE, 2) / ROPE)).astype(np.float64)
    cos_t = np.empty((BPC, 128, 256), np.float32)
    sin_t = np.empty((BPC, 128, 256), np.float32)
    for i in range(BPC):
        nb = (core * BPC + i) % NB
        pos = nb * BS + np.arange(BS, dtype=np.float64)
        ang = pos[None, :] * inv_freq[:, None]          # [16, 256]
        cpat = np.repeat(np.cos(ang), 2, axis=0)        # [32, 256]
        spat = np.repeat(np.sin(ang), 2, axis=0)
        cos_t[i] = np.tile(cpat, (4, 1)).astype(np.float32)
        sin_t[i] = np.tile(spat, (4, 1)).astype(np.float32)
    return cos_t, sin_t


_nc_cache = []


def kernel(inputs_q, inputs_kv, Wq, bq, Wk, bk, Wv, bv, Wo, bo):
    inputs_q = np.asarray(inputs_q, np.float32)
    inputs_kv = np.asarray(inputs_kv, np.float32)
    wq_p, wk_p, wv_c, wo_c, bq_p, bk_p, bo_eff, rt = _host_prep(
        np.asarray(Wq), np.asarray(bq), np.asarray(Wk), np.asarray(bk),
        np.asarray(Wv), np.asarray(bv), np.asarray(Wo), np.asarray(bo),
    )
    xq_all = inputs_q.reshape(BLKS, BS, F)
    xkv_all = inputs_kv.reshape(BLKS, BS, F)

    wq_p = wq_p.astype(WNP)
    wk_p = wk_p.astype(WNP)
    wv_c = wv_c.astype(WNP)
    wo_c = wo_c.astype(WNP)

    if not _nc_cache:
        _nc_cache.append(build_program())
    nc = _nc_cache[0]

    in_maps = []
    for core in range(NCORES):
        cos_t, sin_t = _tables_for_core(core)
        in_maps.append(
            {
                "xq": np.ascontiguousarray(xq_all[core * BPC : (core + 1) * BPC]),
                "xkv": np.ascontiguousarray(xkv_all[core * BPC : (core + 1) * BPC]),
                "wq": wq_p, "wk": wk_p, "wv": wv_c, "wo": wo_c,
                "rt": rt, "bq": bq_p, "bk": bk_p, "bo": bo_eff,
                "ident": np.eye(128, dtype=np.float32),
                "ones": np.ones((1, 16, 64), np.float32),
                "cos": cos_t, "sin": sin_t,
            }
        )
    res = run_bass_kernel_spmd(nc, in_maps, list(range(NCORES)))
    out = np.concatenate([res.results[i]["out"] for i in range(NCORES)], axis=0)
    return out.reshape(B, NB, BS, F)


# revision 16
# speedup vs baseline: 1.7120x; 1.2506x over previous
"""Trainium2 Bass kernel for nn_MultiHeadDotProductAttention_14980845928960.

Block-local multi-head attention with partial RoPE:
  q/k/v projections -> RoPE on first 32 of 64 head dims -> softmax(QK^T/8)V
  -> output projection.  Shapes: inputs [4,16,256,1024], 16 heads x 64 dim,
  blocks of 256 tokens attend locally.

Strategy: data-parallel over the 64 (batch, block) pairs -> 8 blocks/core.
Per block, everything is computed with the contraction dim on SBUF
partitions:
  - x^T via PE transposes; projections as lhsT=W chunk, rhs=x^T chunk.
  - Q/K channel-PERMUTED (host side) so rope dims occupy out-chunks 0-3
    and pass dims chunks 4-7; RoPE = R-matmul (pair swap w/ signs) + two
    elementwise multiplies with cos/sin tables (position tables are
    host-precomputed inputs).
  - scores computed TRANSPOSED (k on partitions) so no P transpose is
    needed; softmax has no max-subtraction (scores ~N(0,1), exp is safe);
    row sums come free as a 65th output row of the PV matmul against
    v_aug = [v | 1] (interleaved 65-col layout); normalization folds into
    the attn PSUM->SBUF evacuation.
  - fp32r (reduced fp32, ~1e-4 rel) matmuls at bf16 rate for N>=256.
All matmul scaling (1/sqrt(D)) and biases fold into host-prepped weights
(bq,bk folded on evac; bv,bo folded as bo_eff = bo + bv @ Wo since
softmax rows sum to one).
"""

import ml_dtypes
import numpy as np

import concourse.bass as bass
import concourse.tile as tile
from concourse import mybir
from concourse.bass_utils import run_bass_kernel_spmd
from concourse.vector_clock import ScopedClock

# ---------------------------------------------------------------- constants
B, NB, BS, F = 4, 16, 256, 1024
H, D, ROPE = 16, 64, 32
NCORES = 8
BLKS = B * NB                 # 64 blocks total
BPC = BLKS // NCORES          # 8 blocks per core
F32 = mybir.dt.float32
F32R = mybir.dt.float32r
BF16 = mybir.dt.bfloat16
WDT = BF16                    # projection-weight / xT / attnT dtype
WNP = ml_dtypes.bfloat16
MULT = mybir.AluOpType.mult
ADD = mybir.AluOpType.add
EXP = mybir.ActivationFunctionType.Exp

# ------------------------------------------------- walrus multi-wait splitter
# This walrus build rejects >1 sync-wait per instruction on several
# instruction structs. Tile attaches several waits to one instruction;
# hoist extras onto NOPs inserted just before it on the same engine.
_split_ctr = [0]


def _split_multi_waits(nc, maxw=1):
    for f in nc.m.functions:
        for bb in f.blocks:
            insts = list(bb.instructions)
            out = []
            changed = False
            for inst in insts:
                si = inst.sync_info
                waits = list(si.on_wait) if si and si.on_wait else []
                if len(waits) > maxw:
                    changed = True
                    for w in waits[:-maxw]:
                        _split_ctr[0] += 1
                        nop = mybir.InstNoOp(
                            name=f"wsplit-{_split_ctr[0]}",
                            ins=[],
                            outs=[],
                            engine=inst.engine,
                        )
                        nop.sync_info = mybir.SyncInfo(on_wait=[w], on_update=[])
                        nc.register_instruction(nop)
                        out.append(nop)
                    si.on_wait = waits[-maxw:]
                out.append(inst)
            if changed:
                bb.instructions = out


def _act_reciprocal(nc, out, in_):
    # ScalarE LUT reciprocal (~1.2e-5 rel, 507ns/[64,256]) -- bass's guard
    # prefers DVE reciprocal, which is 3.3x slower; emit directly.
    eng = nc.scalar
    return eng.add_instruction(
        mybir.InstActivation(
            name=nc.get_next_instruction_name(),
            func=mybir.ActivationFunctionType.Reciprocal,
            ins=[
                eng.lower_ap(in_),
                mybir.ImmediateValue(dtype=F32, value=0.0),
                mybir.ImmediateValue(dtype=F32, value=1.0),
                mybir.ImmediateValue(dtype=F32, value=0.0),
            ],
            outs=[eng.lower_ap(out)],
        )
    )


# ---------------------------------------------------------------- bass build
def _build_block(nc, pools, consts, blk, dram):
    """Emit one (batch, block) tile of work."""
    psum, xin, xt, qk, vpool, ptp, attnp, outp, tabp = pools
    wq_sb, wk_sb, wv_sb, wo_sb, rt_sb, ident, bq_sb, bk_sb, bo_sb, vaug = consts
    xq_d, xkv_d, cos_d, sin_d, out_d = dram

    cos_sb = tabp.tile([128, 256], F32, tag="cos")
    nc.sync.dma_start(out=cos_sb, in_=cos_d[blk])
    sin_sb = tabp.tile([128, 256], F32, tag="sin")
    nc.sync.dma_start(out=sin_sb, in_=sin_d[blk])

    # ---- x^T for both inputs: 8 f-chunks of [128f, 256tok] each
    def transpose_input(x_d, dma_eng):
        # load as 4 [128, 512] tiles (2KB contiguous lines), transpose
        # [128,128] slices out of them
        xt_in = {}
        for t in range(2):
            for fh in range(2):
                xtile = xin.tile([128, 512], F32R, tag="xin", name=f"xin{t}{fh}")
                dma_eng.dma_start(
                    out=xtile,
                    in_=x_d[blk, t * 128 : (t + 1) * 128, fh * 512 : (fh + 1) * 512],
                )
                xt_in[(t, fh)] = xtile
        tiles = []
        for c in range(8):
            ps = psum.tile([128, 256], F32R, tag="ps")
            for t in range(2):
                src = xt_in[(t, c // 4)][:, (c % 4) * 128 : (c % 4 + 1) * 128]
                nc.tensor.transpose(
                    out=ps[:, t * 128 : (t + 1) * 128], in_=src, identity=ident
                )
            tt = xt.tile([128, 256], WDT, tag=f"xt{c}")
            nc.vector.tensor_copy(out=tt, in_=ps)
            tiles.append(tt)
        return tiles


    # ---- Q / K projections (channel-permuted; chunks 0-3 rope, 4-7 pass)
    def qk_proj(w_sb, b_sb, x_tiles, tagpfx):
        outs = []
        for oc in range(8):
            ps = psum.tile([128, 256], F32, tag="ps")
            for c in range(8):
                nc.tensor.matmul(
                    ps,
                    lhsT=w_sb[c][:, oc * 128 : (oc + 1) * 128],
                    rhs=x_tiles[c],
                    start=(c == 0),
                    stop=(c == 7),
                )
            qf = qk.tile([128, 256], F32R, tag=f"{tagpfx}{oc}")
            if oc < 4:
                raw = qk.tile([128, 256], F32R, tag="raw", bufs=2)
                nc.vector.tensor_scalar_add(raw, ps, b_sb[:, oc : oc + 1])
                ps2 = psum.tile([128, 256], F32, tag="ps")
                nc.tensor.matmul(ps2, lhsT=rt_sb, rhs=raw, start=True, stop=True)
                qs2 = qk.tile([128, 256], F32, tag="qs2", bufs=2)
                nc.vector.tensor_tensor(out=qs2, in0=ps2, in1=sin_sb, op=MULT)
                nc.gpsimd.tensor_tensor(out=qf, in0=raw, in1=cos_sb, op=MULT)
                nc.gpsimd.tensor_tensor(out=qf, in0=qf, in1=qs2, op=ADD)
            else:
                nc.vector.tensor_scalar_add(qf, ps, b_sb[:, oc : oc + 1])
            outs.append(qf)
        return outs

    xqT = transpose_input(xq_d, nc.gpsimd)
    qT = qk_proj(wq_sb, bq_sb, xqT, "q")
    xkT = transpose_input(xkv_d, nc.scalar)
    kT = qk_proj(wk_sb, bk_sb, xkT, "k")

    # ---- V projection into interleaved v_aug = [v_h | 1]*64 (128 cols/head)
    # The 64 ones-columns replicate the softmax row-sum onto PV output
    # partitions 64..127, already partition-broadcast for normalization.
    for kc in range(2):
        va = vaug[kc]
        va3 = va.rearrange("p (h c) -> p h c", c=128)
        for b2 in range(2):
            ps = psum.tile([128, 512], F32, tag="ps")
            for c in range(8):
                nc.tensor.matmul(
                    ps,
                    lhsT=xkT[c][:, kc * 128 : (kc + 1) * 128],
                    rhs=wv_sb[c][:, b2 * 512 : (b2 + 1) * 512],
                    start=(c == 0),
                    stop=(c == 7),
                )
            nc.vector.tensor_copy(
                out=va3[:, b2 * 8 : (b2 + 1) * 8, 0:64],
                in_=ps.rearrange("p (h c) -> p h c", c=64),
            )

    # ---- attention (scores transposed: [k, q]; 4 heads per chunk-group)
    attnT = [
        attnp.tile([128, 256], WDT, tag=f"attnT{cc}", name=f"attnT{cc}", bufs=2)
        for cc in range(8)
    ]
    # phase 1: scoresT + exp for all 16 heads (ACT runs exps back-to-back,
    # one table load); phase 2: PV + recip + normalize-evac (one more).
    pts = {}
    for hg in range(4):
        rc, pc = hg, 4 + hg
        for kc in range(2):
            sps = []
            for g in range(4):
                ps = psum.tile([128, 256], F32, tag="ps")
                r0 = 32 * g
                nc.tensor.matmul(
                    ps,
                    lhsT=kT[rc][r0 : r0 + 32, kc * 128 : (kc + 1) * 128],
                    rhs=qT[rc][r0 : r0 + 32, :],
                    start=True,
                    stop=False,
                    tile_position=(r0, 0),
                )
                nc.tensor.matmul(
                    ps,
                    lhsT=kT[pc][r0 : r0 + 32, kc * 128 : (kc + 1) * 128],
                    rhs=qT[pc][r0 : r0 + 32, :],
                    start=False,
                    stop=True,
                    tile_position=(r0, 0),
                )
                sps.append(ps)
            for g in range(4):
                pt = ptp.tile([128, 256], F32R, tag=f"pt{4 * hg + g}_{kc}",
                              name=f"pt{4 * hg + g}_{kc}")
                nc.scalar.activation(out=pt, in_=sps[g], func=EXP)
                pts[(4 * hg + g, kc)] = pt
    for h in range(H):
        aps = psum.tile([128, 256], F32, tag="ps")
        for kc in range(2):
            nc.tensor.matmul(
                aps,
                lhsT=vaug[kc][:, h * 128 : (h + 1) * 128],
                rhs=pts[(h, kc)],
                start=(kc == 0),
                stop=(kc == 1),
            )
        rec_b = attnp.tile([64, 256], F32, tag="recip", bufs=2)
        _act_reciprocal(nc, rec_b, aps[64:128, :])
        cc, r0 = h // 2, (h % 2) * 64
        nc.vector.tensor_tensor(
            out=attnT[cc][r0 : r0 + 64, :],
            in0=aps[0:64, :],
            in1=rec_b,
            op=MULT,
        )

    # ---- output projection + bias
    for t2 in range(2):
        for n2 in range(2):
            ps = psum.tile([128, 512], F32, tag="ps")
            for cc in range(8):
                nc.tensor.matmul(
                    ps,
                    lhsT=attnT[cc][:, t2 * 128 : (t2 + 1) * 128],
                    rhs=wo_sb[cc][:, n2 * 512 : (n2 + 1) * 512],
                    start=(cc == 0),
                    stop=(cc == 7),
                )
            ob = outp.tile([128, 512], F32, tag="outsb")
            nc.vector.tensor_tensor(
                out=ob,
                in0=ps,
                in1=bo_sb[:, n2 * 512 : (n2 + 1) * 512],
                op=ADD,
            )
            nc.sync.dma_start(
                out=out_d[blk, t2 * 128 : (t2 + 1) * 128, n2 * 512 : (n2 + 1) * 512],
                in_=ob,
            )


def build_program():
    nc = bass.Bass("TRN2")
    xq_d = nc.dram_tensor("xq", [BPC, BS, F], F32R, kind="ExternalInput")
    xkv_d = nc.dram_tensor("xkv", [BPC, BS, F], F32R, kind="ExternalInput")
    wq_d = nc.dram_tensor("wq", [8, 128, F], WDT, kind="ExternalInput")
    wk_d = nc.dram_tensor("wk", [8, 128, F], WDT, kind="ExternalInput")
    wv_d = nc.dram_tensor("wv", [8, 128, F], WDT, kind="ExternalInput")
    wo_d = nc.dram_tensor("wo", [8, 128, F], WDT, kind="ExternalInput")
    rt_d = nc.dram_tensor("rt", [128, 128], F32R, kind="ExternalInput")
    ident_d = nc.dram_tensor("ident", [128, 128], F32R, kind="ExternalInput")
    ones_d = nc.dram_tensor("ones", [1, 16, 64], F32R, kind="ExternalInput")
    bq_d = nc.dram_tensor("bq", [128, 8], F32, kind="ExternalInput")
    bk_d = nc.dram_tensor("bk", [128, 8], F32, kind="ExternalInput")
    bo_d = nc.dram_tensor("bo", [1, F], F32, kind="ExternalInput")
    cos_d = nc.dram_tensor("cos", [BPC, 128, 256], F32, kind="ExternalInput")
    sin_d = nc.dram_tensor("sin", [BPC, 128, 256], F32, kind="ExternalInput")
    out_d = nc.dram_tensor("out", [BPC, BS, F], F32, kind="ExternalOutput")

    with tile.TileContext(nc) as tc:
        with (
            tc.tile_pool(name="wpool", bufs=1) as wpool,
            tc.tile_pool(name="psum", bufs=8, space="PSUM") as psum,
            tc.tile_pool(name="xin", bufs=4) as xin,
            tc.tile_pool(name="xt", bufs=1) as xt,
            tc.tile_pool(name="qk", bufs=2) as qk,
            tc.tile_pool(name="vpool", bufs=1) as vpool,
            tc.tile_pool(name="ptp", bufs=1) as ptp,
            tc.tile_pool(name="attnp", bufs=1) as attnp,
            tc.tile_pool(name="outp", bufs=2) as outp,
            tc.tile_pool(name="tabp", bufs=2) as tabp,
        ):
            def wtiles(src, tagpfx):
                ts = []
                for c in range(8):
                    t = wpool.tile([128, F], WDT, tag=f"{tagpfx}{c}")
                    nc.sync.dma_start(out=t, in_=src[c])
                    ts.append(t)
                return ts

            wq_sb = wtiles(wq_d, "wq")
            wk_sb = wtiles(wk_d, "wk")
            wv_sb = wtiles(wv_d, "wv")
            wo_sb = wtiles(wo_d, "wo")
            rt_sb = wpool.tile([128, 128], F32R, tag="rt")
            nc.sync.dma_start(out=rt_sb, in_=rt_d[:])
            ident = wpool.tile([128, 128], F32R, tag="ident")
            nc.sync.dma_start(out=ident, in_=ident_d[:])
            bq_sb = wpool.tile([128, 8], F32, tag="bq")
            nc.sync.dma_start(out=bq_sb, in_=bq_d[:])
            bk_sb = wpool.tile([128, 8], F32, tag="bk")
            nc.sync.dma_start(out=bk_sb, in_=bk_d[:])
            bo_sb = wpool.tile([128, F], F32, tag="bo")
            nc.sync.dma_start(out=bo_sb, in_=bo_d[0:1, :].to_broadcast([128, F]))

            vaug = []
            for kc in range(2):
                va = vpool.tile([128, 2048], F32R, tag=f"vaug{kc}", name=f"vaug{kc}")
                nc.sync.dma_start(
                    out=va.rearrange("p (h c) -> p h c", c=128)[:, :, 64:128],
                    in_=ones_d[:].to_broadcast([128, 16, 64]),
                )
                vaug.append(va)

            pools = (psum, xin, xt, qk, vpool, ptp, attnp, outp, tabp)
            consts = (wq_sb, wk_sb, wv_sb, wo_sb, rt_sb, ident, bq_sb, bk_sb, bo_sb, vaug)
            dram = (xq_d, xkv_d, cos_d, sin_d, out_d)
            for blk in range(BPC):
                _build_block(nc, pools, consts, blk, dram)

    _split_multi_waits(nc)
    return nc


# ---------------------------------------------------------------- host side
def _host_prep(Wq, bq, Wk, bk, Wv, bv, Wo, bo):
    """Permute/scale weights; fold biases."""
    old_of_new = np.empty(F, np.int64)
    for h in range(H):
        old_of_new[h * ROPE : (h + 1) * ROPE] = h * D + np.arange(ROPE)
        old_of_new[512 + h * ROPE : 512 + (h + 1) * ROPE] = (
            h * D + ROPE + np.arange(ROPE)
        )
    wq_flat = (Wq.reshape(F, F) / np.sqrt(D)).astype(np.float32)
    wq_p = np.ascontiguousarray(wq_flat[:, old_of_new]).reshape(8, 128, F)
    wk_flat = Wk.reshape(F, F).astype(np.float32)
    wk_p = np.ascontiguousarray(wk_flat[:, old_of_new]).reshape(8, 128, F)
    wv_c = np.ascontiguousarray(Wv.reshape(F, F)).reshape(8, 128, F)
    wo_c = np.ascontiguousarray(Wo.reshape(F, F)).reshape(8, 128, F)
    bq_p = np.ascontiguousarray(
        (bq.reshape(F) / np.sqrt(D))[old_of_new].reshape(8, 128).T
    ).astype(np.float32)
    bk_p = np.ascontiguousarray(bk.reshape(F)[old_of_new].reshape(8, 128).T).astype(
        np.float32
    )
    bo_eff = (bo + bv.reshape(F) @ Wo.reshape(F, F)).reshape(1, F).astype(np.float32)

    # R^T for rotate_every_two with signs: (R@q)[2i] = -q[2i+1]; [2i+1] = q[2i]
    R = np.zeros((128, 128), np.float32)
    for g in range(4):          # 4 heads per rope chunk, 32 rows each
        for i in range(ROPE // 2):
            R[g * 32 + 2 * i, g * 32 + 2 * i + 1] = -1.0
            R[g * 32 + 2 * i + 1, g * 32 + 2 * i] = 1.0
    rt = np.ascontiguousarray(R.T)
    return wq_p, wk_p, wv_c, wo_c, bq_p, bk_p, bo_eff, rt


def _tables_for_core(core):
    """cos/sin tables [BPC, 128, 256] for this core's blocks."""
    inv_freq = (1.0 / 10000.0 ** (np.arange(0, ROPE, 2) / ROPE)).astype(np.float64)
    cos_t = np.empty((BPC, 128, 256), np.float32)
    sin_t = np.empty((BPC, 128, 256), np.float32)
    for i in range(BPC):
        nb = (core * BPC + i) % NB
        pos = nb * BS + np.arange(BS, dtype=np.float64)
        ang = pos[None, :] * inv_freq[:, None]          # [16, 256]
        cpat = np.repeat(np.cos(ang), 2, axis=0)        # [32, 256]
        spat = np.repeat(np.sin(ang), 2, axis=0)
        cos_t[i] = np.tile(cpat, (4, 1)).astype(np.float32)
        sin_t[i] = np.tile(spat, (4, 1)).astype(np.float32)
    return cos_t, sin_t


_nc_cache = []


def kernel(inputs_q, inputs_kv, Wq, bq, Wk, bk, Wv, bv, Wo, bo):
    inputs_q = np.asarray(inputs_q, np.float32)
    inputs_kv = np.asarray(inputs_kv, np.float32)
    wq_p, wk_p, wv_c, wo_c, bq_p, bk_p, bo_eff, rt = _host_prep(
        np.asarray(Wq), np.asarray(bq), np.asarray(Wk), np.asarray(bk),
        np.asarray(Wv), np.asarray(bv), np.asarray(Wo), np.asarray(bo),
    )
    xq_all = inputs_q.reshape(BLKS, BS, F)
    xkv_all = inputs_kv.reshape(BLKS, BS, F)

    wq_p = wq_p.astype(WNP)
    wk_p = wk_p.astype(WNP)
    wv_c = wv_c.astype(WNP)
    wo_c = wo_c.astype(WNP)

    if not _nc_cache:
        _nc_cache.append(build_program())
    nc = _nc_cache[0]

    in_maps = []
    for core in range(NCORES):
        cos_t, sin_t = _tables_for_core(core)
        in_maps.append(
            {
                "xq": np.ascontiguousarray(xq_all[core * BPC : (core + 1) * BPC]),
                "xkv": np.ascontiguousarray(xkv_all[core * BPC : (core + 1) * BPC]),
                "wq": wq_p, "wk": wk_p, "wv": wv_c, "wo": wo_c,
                "rt": rt, "bq": bq_p, "bk": bk_p, "bo": bo_eff,
                "ident": np.eye(128, dtype=np.float32),
                "ones": np.ones((1, 16, 64), np.float32),
                "cos": cos_t, "sin": sin_t,
            }
        )
    res = run_bass_kernel_spmd(nc, in_maps, list(range(NCORES)))
    out = np.concatenate([res.results[i]["out"] for i in range(NCORES)], axis=0)
    return out.reshape(B, NB, BS, F)
